# revision 25
# baseline (speedup 1.0000x reference)
"""DifferentiableLogicLayer Trainium2 kernel.

Math: reference computes, per batch row t and gate g (G = INPUT_SIZE = 8192):
    a = x[t, g], b = x[t, (g+1) % 8192]            (x uniform in [0,1] -> clip no-op)
    out[t, g] = sum_o softmax(gate_logits[g])_o * op_o(a, b)
Each of the 16 soft ops is linear in {1, a, b, ab}, so with probs p:
    out = C0 + CA*a + CB*b + CAB*a*b
    C0  = p8+..+p15
    CA  = p2+p3+p6+p7-p8-p9-p12-p13
    CB  = p4+p5+p6+p7-p8-p9-p10-p11
    CAB = p1-p2-p4-2*p6-p7+p8+2*p9+p11+p13-p14
Factored: out = ((CAB*a + CB)*b) + (CA*a + C0)  -> 6 elementwise passes.

Sharding: gates across the 8 cores (1024 each; gates are independent, each
needs x columns [g, g+1]).  Per-core inputs:
    xs [2048, 1025] = x cols [1024c .. 1024c+1024] (halo col, wraparound)
    gl [1024, 16]   = gate_logits rows for this core's gates

Coefficient prep runs in a [128 partitions, 8 gates x 16 ops] layout (exp on
ScalarE, subset reductions + combines on VectorE, all on 8-element frees so
they cost ~0.1us each), then each [128, 8] coefficient is reshaped to a
[1, 1024] row by a small SBUF->SBUF DMA and broadcast to a [128, G] PSUM tile
with K=1 matmuls (ones x row).  CAB/CB are finalized first so the main loop
starts as early as possible.

Engine assignment (measured port-sharing rule: GPSIMD's SBUF port is
VectorE's rd1, so GP only contends with DVE instructions whose BOTH tensor
operands live in SBUF — and DVE/GP running 2-port-DVE + GP concurrently is
net-negative):
    VectorE: u = a*R_cab, u += R_cb, v = a*R_ca, v += R_c0   (rd0 + PSUM)
    GPSIMD:  w = u*b, o = w + v                              (pure SBUF)
VectorE runs MEGA=2 batch tiles per instruction (3D APs + step-0 broadcast on
the coefficient operand) to amortize fixed costs; GPSIMD keeps flat 2D
per-subtile APs (3D APs are ~20% slower on the Q7s).
"""

import numpy as np

NUM_GATES = 8192
INPUT_SIZE = 8192
BATCH = 2048
N_CORES = 8
G = NUM_GATES // N_CORES  # 1024 local gates
P = 128
MEGA = 2

_CACHE = {}


def _build_nc(reps=1, mega=MEGA, warm=False, rows_on_act=False, substore=False, bulk_on_act=False, inplace_o=False, first1=True, xb=4, uvb=4, wob=3):
    from contextlib import ExitStack

    import concourse.bacc as bacc
    import concourse.mybir as mybir
    from concourse.mybir import AluOpType as Op
    from concourse.tile import TileContext

    f32 = mybir.dt.float32
    Ax = mybir.AxisListType
    Act = mybir.ActivationFunctionType

    nc = bacc.Bacc("TRN2", target_bir_lowering=False, debug=False,
                   num_devices=N_CORES)
    xs = nc.dram_tensor("xs", [BATCH, G + 1], f32, kind="ExternalInput").ap()
    gl = nc.dram_tensor("gl", [G, 16], f32, kind="ExternalInput").ap()
    out = nc.dram_tensor("out", [BATCH, G], f32, kind="ExternalOutput").ap()

    with TileContext(nc) as tc, ExitStack() as ctx:
        cpool = ctx.enter_context(tc.tile_pool(name="coef", bufs=1))
        rpool = ctx.enter_context(tc.tile_pool(name="rows", bufs=1))
        ppool = ctx.enter_context(tc.tile_pool(name="psum", bufs=1, space="PSUM"))
        xpool = ctx.enter_context(tc.tile_pool(name="x", bufs=xb))
        upool = ctx.enter_context(tc.tile_pool(name="tu", bufs=uvb))
        vpool = ctx.enter_context(tc.tile_pool(name="tv", bufs=uvb))
        wpool = ctx.enter_context(tc.tile_pool(name="tw", bufs=wob))
        opool = ctx.enter_context(tc.tile_pool(name="o", bufs=wob))

        row_dma = nc.scalar.dma_start if rows_on_act else nc.sync.dma_start
        bulk_dma = nc.scalar.dma_start if bulk_on_act else nc.sync.dma_start

        for rep in range(reps):
            # ---- coefficients in [128 partitions, 8 gates x 16 ops] ----
            lg = cpool.tile([P, 8 * 16], f32, name=f"lg{rep}")
            row_dma(out=lg[:, :], in_=gl.rearrange("(p n) o -> p (n o)", p=P))
            E = cpool.tile([P, 8 * 16], f32, name=f"E{rep}")
            nc.scalar.activation(E[:, :], lg[:, :], Act.Exp)
            E3 = E[:, :].rearrange("p (n o) -> p n o", o=16)

            def red(sl, name):
                t = cpool.tile([P, 8], f32, name=name)
                nc.vector.tensor_reduce(t[:, :], sl, Ax.X, Op.add)
                return t

            def Eo(o):
                return E3[:, :, o]

            den = red(E3[:, :, 0:16], f"den{rep}")
            rden = cpool.tile([P, 8], f32, name=f"rden{rep}")
            nc.vector.reciprocal(rden[:, :], den[:, :])

            ones = rpool.tile([1, P], f32, name=f"ones{rep}")
            nc.vector.memset(ones[:, :], 1.0)

            R = {nm: ppool.tile([P, G], f32, name=f"R_{nm}{rep}")
                 for nm in ("cab", "cb", "ca", "c0")}
            if warm:
                nc.tensor.matmul(R["c0"][:, 0:P], ones[:, :], ones[:, :],
                                 start=True, stop=True)

            def finalize(nm, numer):
                c = cpool.tile([P, 8], f32, name=f"c_{nm}{rep}")
                nc.vector.tensor_tensor(c[:, :], numer[:, :], rden[:, :], Op.mult)
                row = rpool.tile([1, G], f32, name=f"row_{nm}{rep}")
                row_dma(out=row[:, :], in_=c[:, :])
                for j in range(0, G, 512):
                    nc.tensor.matmul(R[nm][:, j:j + 512], ones[:, :],
                                     row[:, j:j + 512], start=True, stop=True)

            # CAB = p1-p2-p4-2*p6-p7+p8+2*p9+p11+p13-p14  (needed first)
            nab = cpool.tile([P, 8], f32, name=f"nab{rep}")
            nc.vector.scalar_tensor_tensor(nab[:, :], Eo(6), -2.0, Eo(1), Op.mult, Op.add)
            t2 = cpool.tile([P, 8], f32, name=f"t2{rep}")
            nc.vector.scalar_tensor_tensor(t2[:, :], Eo(9), 2.0, Eo(8), Op.mult, Op.add)
            nc.vector.tensor_tensor(nab[:, :], nab[:, :], t2[:, :], Op.add)
            nc.vector.tensor_tensor(t2[:, :], Eo(11), Eo(13), Op.add)
            nc.vector.tensor_tensor(nab[:, :], nab[:, :], t2[:, :], Op.add)
            nc.vector.tensor_tensor(t2[:, :], Eo(2), Eo(4), Op.add)
            nc.vector.tensor_tensor(t2[:, :], t2[:, :], Eo(7), Op.add)
            nc.vector.tensor_tensor(t2[:, :], t2[:, :], Eo(14), Op.add)
            nc.vector.tensor_tensor(nab[:, :], nab[:, :], t2[:, :], Op.subtract)
            finalize("cab", nab)

            # CB = p4+p5+p6+p7-p8-p9-p10-p11 (second: completes u-chain inputs)
            pb1 = red(E3[:, :, 4:8], f"pb1{rep}")
            pb2 = red(E3[:, :, 8:12], f"pb2{rep}")
            nb = cpool.tile([P, 8], f32, name=f"nb{rep}")
            nc.vector.tensor_tensor(nb[:, :], pb1[:, :], pb2[:, :], Op.subtract)
            finalize("cb", nb)

            # CA = p2+p3+p6+p7-p8-p9-p12-p13
            pa1 = red(E3[:, :, 2:4], f"pa1{rep}")
            pa2 = red(E3[:, :, 6:8], f"pa2{rep}")
            pa3 = red(E3[:, :, 8:10], f"pa3{rep}")
            pa4 = red(E3[:, :, 12:14], f"pa4{rep}")
            na = cpool.tile([P, 8], f32, name=f"na{rep}")
            nc.vector.tensor_tensor(na[:, :], pa1[:, :], pa2[:, :], Op.add)
            nc.vector.tensor_tensor(na[:, :], na[:, :], pa3[:, :], Op.subtract)
            nc.vector.tensor_tensor(na[:, :], na[:, :], pa4[:, :], Op.subtract)
            finalize("ca", na)

            # C0 = p8+..+p15
            n0 = red(E3[:, :, 8:16], f"n0{rep}")
            finalize("c0", n0)

            def bc(r, m):
                return r[:, :].unsqueeze(1).broadcast_to([P, m, G])

            # ---- main loop ----
            if first1:
                sizes = [1] + [mega] * ((BATCH // P - 2) // mega) + [1]
            else:
                sizes = [mega] * (BATCH // (P * mega))
            assert sum(sizes) == BATCH // P
            rows_lo = 0
            for gi, m in enumerate(sizes):
                xin = xs[rows_lo:rows_lo + P * m, :].rearrange(
                    "(m p) c -> p m c", m=m)
                rows_next = rows_lo + P * m
                xt = xpool.tile([P, m, G + 1], f32, name=f"xt{rep}_{gi}", tag="xt")
                bulk_dma(out=xt[:, :, :], in_=xin)
                a = xt[:, :, 0:G]

                u = upool.tile([P, m, G], f32, name=f"u{rep}_{gi}", tag="u")
                v = vpool.tile([P, m, G], f32, name=f"v{rep}_{gi}", tag="v")
                nc.vector.tensor_tensor(u[:, :, :], a, bc(R["cab"], m), Op.mult)
                nc.vector.tensor_tensor(u[:, :, :], u[:, :, :], bc(R["cb"], m), Op.add)
                nc.vector.tensor_tensor(v[:, :, :], a, bc(R["ca"], m), Op.mult)
                nc.vector.tensor_tensor(v[:, :, :], v[:, :, :], bc(R["c0"], m), Op.add)

                w = wpool.tile([P, m, G], f32, name=f"w{rep}_{gi}", tag="w")
                o = w if inplace_o else opool.tile([P, m, G], f32,
                                                   name=f"o{rep}_{gi}", tag="o")
                for sm in range(m):
                    nc.gpsimd.tensor_tensor(w[:, sm, :], u[:, sm, :],
                                            xt[:, sm, 1:G + 1], Op.mult)
                    nc.gpsimd.tensor_tensor(o[:, sm, :], w[:, sm, :],
                                            v[:, sm, :], Op.add)
                    if substore:
                        nc.sync.dma_start(
                            out=out[rows_lo + sm * P:rows_lo + (sm + 1) * P, :],
                            in_=o[:, sm, :])
                if not substore:
                    oout = out[rows_lo:rows_lo + P * m, :].rearrange(
                        "(m p) c -> p m c", m=m)
                    nc.sync.dma_start(out=oout, in_=o[:, :, :])
                rows_lo = rows_next

    nc.compile()
    return nc


def _get_nc(reps=1, **kw):
    key = (reps, tuple(sorted(kw.items())))
    if key not in _CACHE:
        _CACHE[key] = _build_nc(reps, **kw)
    return _CACHE[key]


def _shard_inputs(x, gate_logits):
    x = np.ascontiguousarray(x, dtype=np.float32)
    gate_logits = np.ascontiguousarray(gate_logits, dtype=np.float32)
    xs_full = np.concatenate([x, x[:, :1]], axis=1)  # wraparound halo
    in_maps = []
    for c in range(N_CORES):
        in_maps.append({
            "xs": np.ascontiguousarray(xs_full[:, c * G:c * G + G + 1]),
            "gl": np.ascontiguousarray(gate_logits[c * G:(c + 1) * G]),
        })
    return in_maps


def kernel(x, gate_logits):
    from concourse.bass_utils import run_bass_kernel_spmd

    nc = _get_nc()
    in_maps = _shard_inputs(x, gate_logits)
    res = run_bass_kernel_spmd(nc, in_maps, core_ids=list(range(N_CORES)))
    return np.concatenate([res.results[c]["out"] for c in range(N_CORES)], axis=1)


# revision 27
# speedup vs baseline: 1.0071x; 1.0071x over previous
"""DifferentiableLogicLayer Trainium2 kernel.

Math: reference computes, per batch row t and gate g (G = INPUT_SIZE = 8192):
    a = x[t, g], b = x[t, (g+1) % 8192]            (x uniform in [0,1] -> clip no-op)
    out[t, g] = sum_o softmax(gate_logits[g])_o * op_o(a, b)
Each of the 16 soft ops is linear in {1, a, b, ab}, so with probs p:
    out = C0 + CA*a + CB*b + CAB*a*b
    C0  = p8+..+p15
    CA  = p2+p3+p6+p7-p8-p9-p12-p13
    CB  = p4+p5+p6+p7-p8-p9-p10-p11
    CAB = p1-p2-p4-2*p6-p7+p8+2*p9+p11+p13-p14
Factored: out = ((CAB*a + CB)*b) + (CA*a + C0)  -> 6 elementwise passes.

Sharding: gates across the 8 cores (1024 each; gates are independent, each
needs x columns [g, g+1]).  Per-core inputs:
    xs [2048, 1025] = x cols [1024c .. 1024c+1024] (halo col, wraparound)
    gl [1024, 16]   = gate_logits rows for this core's gates

Coefficient prep runs in a [128 partitions, 8 gates x 16 ops] layout (exp on
ScalarE, subset reductions + combines on VectorE, all on 8-element frees so
they cost ~0.1us each), then each [128, 8] coefficient is reshaped to a
[1, 1024] row by a small SBUF->SBUF DMA and broadcast to a [128, G] PSUM tile
with K=1 matmuls (ones x row).  CAB/CB are finalized first so the main loop
starts as early as possible.

Engine assignment (measured port-sharing rule: GPSIMD's SBUF port is
VectorE's rd1, so GP only contends with DVE instructions whose BOTH tensor
operands live in SBUF — and DVE/GP running 2-port-DVE + GP concurrently is
net-negative):
    VectorE: u = a*R_cab, u += R_cb, v = a*R_ca, v += R_c0   (rd0 + PSUM)
    GPSIMD:  w = u*b, o = w + v                              (pure SBUF)
VectorE runs MEGA=2 batch tiles per instruction (3D APs + step-0 broadcast on
the coefficient operand) to amortize fixed costs; GPSIMD keeps flat 2D
per-subtile APs (3D APs are ~20% slower on the Q7s).
"""

import numpy as np

NUM_GATES = 8192
INPUT_SIZE = 8192
BATCH = 2048
N_CORES = 8
G = NUM_GATES // N_CORES  # 1024 local gates
P = 128
MEGA = 2

_CACHE = {}


def _build_nc(reps=1, mega=MEGA, warm=False, rows_on_act=False, substore=False, bulk_on_act=False, inplace_o=False, first1=True, xb=4, uvb=4, wob=3, chunk0=False):
    from contextlib import ExitStack

    import concourse.bacc as bacc
    import concourse.mybir as mybir
    from concourse.mybir import AluOpType as Op
    from concourse.tile import TileContext

    f32 = mybir.dt.float32
    Ax = mybir.AxisListType
    Act = mybir.ActivationFunctionType

    nc = bacc.Bacc("TRN2", target_bir_lowering=False, debug=False,
                   num_devices=N_CORES)
    xs = nc.dram_tensor("xs", [BATCH, G + 1], f32, kind="ExternalInput").ap()
    gl = nc.dram_tensor("gl", [G, 16], f32, kind="ExternalInput").ap()
    out = nc.dram_tensor("out", [BATCH, G], f32, kind="ExternalOutput").ap()

    with TileContext(nc) as tc, ExitStack() as ctx:
        cpool = ctx.enter_context(tc.tile_pool(name="coef", bufs=1))
        rpool = ctx.enter_context(tc.tile_pool(name="rows", bufs=1))
        ppool = ctx.enter_context(tc.tile_pool(name="psum", bufs=1, space="PSUM"))
        xpool = ctx.enter_context(tc.tile_pool(name="x", bufs=xb))
        upool = ctx.enter_context(tc.tile_pool(name="tu", bufs=uvb))
        vpool = ctx.enter_context(tc.tile_pool(name="tv", bufs=uvb))
        wpool = ctx.enter_context(tc.tile_pool(name="tw", bufs=wob))
        opool = ctx.enter_context(tc.tile_pool(name="o", bufs=wob))

        row_dma = nc.scalar.dma_start if rows_on_act else nc.sync.dma_start
        bulk_dma = nc.scalar.dma_start if bulk_on_act else nc.sync.dma_start

        for rep in range(reps):
            # ---- coefficients in [128 partitions, 8 gates x 16 ops] ----
            lg = cpool.tile([P, 8 * 16], f32, name=f"lg{rep}")
            row_dma(out=lg[:, :], in_=gl.rearrange("(p n) o -> p (n o)", p=P))
            E = cpool.tile([P, 8 * 16], f32, name=f"E{rep}")
            nc.scalar.activation(E[:, :], lg[:, :], Act.Exp)
            E3 = E[:, :].rearrange("p (n o) -> p n o", o=16)

            def red(sl, name):
                t = cpool.tile([P, 8], f32, name=name)
                nc.vector.tensor_reduce(t[:, :], sl, Ax.X, Op.add)
                return t

            def Eo(o):
                return E3[:, :, o]

            den = red(E3[:, :, 0:16], f"den{rep}")
            rden = cpool.tile([P, 8], f32, name=f"rden{rep}")
            nc.vector.reciprocal(rden[:, :], den[:, :])

            ones = rpool.tile([1, P], f32, name=f"ones{rep}")
            nc.vector.memset(ones[:, :], 1.0)

            R = {nm: ppool.tile([P, G], f32, name=f"R_{nm}{rep}")
                 for nm in ("cab", "cb", "ca", "c0")}
            if warm:
                nc.tensor.matmul(R["c0"][:, 0:P], ones[:, :], ones[:, :],
                                 start=True, stop=True)

            def finalize(nm, numer):
                c = cpool.tile([P, 8], f32, name=f"c_{nm}{rep}")
                nc.vector.tensor_tensor(c[:, :], numer[:, :], rden[:, :], Op.mult)
                row = rpool.tile([1, G], f32, name=f"row_{nm}{rep}")
                row_dma(out=row[:, :], in_=c[:, :])
                for j in range(0, G, 512):
                    nc.tensor.matmul(R[nm][:, j:j + 512], ones[:, :],
                                     row[:, j:j + 512], start=True, stop=True)

            # CAB = p1-p2-p4-2*p6-p7+p8+2*p9+p11+p13-p14  (needed first)
            nab = cpool.tile([P, 8], f32, name=f"nab{rep}")
            nc.vector.scalar_tensor_tensor(nab[:, :], Eo(6), -2.0, Eo(1), Op.mult, Op.add)
            t2 = cpool.tile([P, 8], f32, name=f"t2{rep}")
            nc.vector.scalar_tensor_tensor(t2[:, :], Eo(9), 2.0, Eo(8), Op.mult, Op.add)
            nc.vector.tensor_tensor(nab[:, :], nab[:, :], t2[:, :], Op.add)
            nc.vector.tensor_tensor(t2[:, :], Eo(11), Eo(13), Op.add)
            nc.vector.tensor_tensor(nab[:, :], nab[:, :], t2[:, :], Op.add)
            nc.vector.tensor_tensor(t2[:, :], Eo(2), Eo(4), Op.add)
            nc.vector.tensor_tensor(t2[:, :], t2[:, :], Eo(7), Op.add)
            nc.vector.tensor_tensor(t2[:, :], t2[:, :], Eo(14), Op.add)
            nc.vector.tensor_tensor(nab[:, :], nab[:, :], t2[:, :], Op.subtract)
            finalize("cab", nab)

            # CB = p4+p5+p6+p7-p8-p9-p10-p11 (second: completes u-chain inputs)
            pb1 = red(E3[:, :, 4:8], f"pb1{rep}")
            pb2 = red(E3[:, :, 8:12], f"pb2{rep}")
            nb = cpool.tile([P, 8], f32, name=f"nb{rep}")
            nc.vector.tensor_tensor(nb[:, :], pb1[:, :], pb2[:, :], Op.subtract)
            finalize("cb", nb)

            # CA = p2+p3+p6+p7-p8-p9-p12-p13
            pa1 = red(E3[:, :, 2:4], f"pa1{rep}")
            pa2 = red(E3[:, :, 6:8], f"pa2{rep}")
            pa3 = red(E3[:, :, 8:10], f"pa3{rep}")
            pa4 = red(E3[:, :, 12:14], f"pa4{rep}")
            na = cpool.tile([P, 8], f32, name=f"na{rep}")
            nc.vector.tensor_tensor(na[:, :], pa1[:, :], pa2[:, :], Op.add)
            nc.vector.tensor_tensor(na[:, :], na[:, :], pa3[:, :], Op.subtract)
            nc.vector.tensor_tensor(na[:, :], na[:, :], pa4[:, :], Op.subtract)
            finalize("ca", na)

            # C0 = p8+..+p15
            n0 = red(E3[:, :, 8:16], f"n0{rep}")
            finalize("c0", n0)

            def bc(r, m):
                return r[:, :].unsqueeze(1).broadcast_to([P, m, G])

            # ---- main loop ----
            if chunk0:
                sizes = [1, 1] + [mega] * ((BATCH // P - 4) // mega) + [1, 1]
            elif first1:
                sizes = [1] + [mega] * ((BATCH // P - 2) // mega) + [1]
            else:
                sizes = [mega] * (BATCH // (P * mega))
            assert sum(sizes) == BATCH // P
            rows_lo = 0
            for gi, m in enumerate(sizes):
                xin = xs[rows_lo:rows_lo + P * m, :].rearrange(
                    "(m p) c -> p m c", m=m)
                rows_next = rows_lo + P * m
                xt = xpool.tile([P, m, G + 1], f32, name=f"xt{rep}_{gi}", tag="xt")
                bulk_dma(out=xt[:, :, :], in_=xin)
                a = xt[:, :, 0:G]

                u = upool.tile([P, m, G], f32, name=f"u{rep}_{gi}", tag="u")
                v = vpool.tile([P, m, G], f32, name=f"v{rep}_{gi}", tag="v")
                w = wpool.tile([P, m, G], f32, name=f"w{rep}_{gi}", tag="w")
                o = w if inplace_o else opool.tile([P, m, G], f32,
                                                   name=f"o{rep}_{gi}", tag="o")
                if chunk0 and gi < 2:
                    # group 0 in 512-col halves: each half depends only on the
                    # matching 512-col broadcast chunks, so the GPSIMD stream
                    # starts ~4us earlier
                    x2, u2, v2 = xt[:, 0, :], u[:, 0, :], v[:, 0, :]
                    w2, o2 = w[:, 0, :], o[:, 0, :]
                    for h in (0, 512):
                        hs = slice(h, h + 512)
                        nc.vector.tensor_tensor(u2[:, hs], x2[:, hs],
                                                R["cab"][:, hs], Op.mult)
                        nc.vector.tensor_tensor(u2[:, hs], u2[:, hs],
                                                R["cb"][:, hs], Op.add)
                        nc.vector.tensor_tensor(v2[:, hs], x2[:, hs],
                                                R["ca"][:, hs], Op.mult)
                        nc.vector.tensor_tensor(v2[:, hs], v2[:, hs],
                                                R["c0"][:, hs], Op.add)
                        nc.gpsimd.tensor_tensor(w2[:, hs], u2[:, hs],
                                                x2[:, h + 1:h + 513], Op.mult)
                        nc.gpsimd.tensor_tensor(o2[:, hs], w2[:, hs],
                                                v2[:, hs], Op.add)
                else:
                    nc.vector.tensor_tensor(u[:, :, :], a, bc(R["cab"], m), Op.mult)
                    nc.vector.tensor_tensor(u[:, :, :], u[:, :, :], bc(R["cb"], m), Op.add)
                    nc.vector.tensor_tensor(v[:, :, :], a, bc(R["ca"], m), Op.mult)
                    nc.vector.tensor_tensor(v[:, :, :], v[:, :, :], bc(R["c0"], m), Op.add)
                    for sm in range(m):
                        nc.gpsimd.tensor_tensor(w[:, sm, :], u[:, sm, :],
                                                xt[:, sm, 1:G + 1], Op.mult)
                        nc.gpsimd.tensor_tensor(o[:, sm, :], w[:, sm, :],
                                                v[:, sm, :], Op.add)
                if substore:
                    for sm in range(m):
                        nc.sync.dma_start(
                            out=out[rows_lo + sm * P:rows_lo + (sm + 1) * P, :],
                            in_=o[:, sm, :])
                if not substore:
                    oout = out[rows_lo:rows_lo + P * m, :].rearrange(
                        "(m p) c -> p m c", m=m)
                    nc.sync.dma_start(out=oout, in_=o[:, :, :])
                rows_lo = rows_next

    nc.compile()
    return nc


def _get_nc(reps=1, **kw):
    key = (reps, tuple(sorted(kw.items())))
    if key not in _CACHE:
        _CACHE[key] = _build_nc(reps, **kw)
    return _CACHE[key]


def _shard_inputs(x, gate_logits):
    x = np.ascontiguousarray(x, dtype=np.float32)
    gate_logits = np.ascontiguousarray(gate_logits, dtype=np.float32)
    xs_full = np.concatenate([x, x[:, :1]], axis=1)  # wraparound halo
    in_maps = []
    for c in range(N_CORES):
        in_maps.append({
            "xs": np.ascontiguousarray(xs_full[:, c * G:c * G + G + 1]),
            "gl": np.ascontiguousarray(gate_logits[c * G:(c + 1) * G]),
        })
    return in_maps


def kernel(x, gate_logits):
    from concourse.bass_utils import run_bass_kernel_spmd

    nc = _get_nc()
    in_maps = _shard_inputs(x, gate_logits)
    res = run_bass_kernel_spmd(nc, in_maps, core_ids=list(range(N_CORES)))
    return np.concatenate([res.results[c]["out"] for c in range(N_CORES)], axis=1)


# revision 29
# speedup vs baseline: 1.0075x; 1.0004x over previous
"""DifferentiableLogicLayer Trainium2 kernel.

Math: reference computes, per batch row t and gate g (G = INPUT_SIZE = 8192):
    a = x[t, g], b = x[t, (g+1) % 8192]            (x uniform in [0,1] -> clip no-op)
    out[t, g] = sum_o softmax(gate_logits[g])_o * op_o(a, b)
Each of the 16 soft ops is linear in {1, a, b, ab}, so with probs p:
    out = C0 + CA*a + CB*b + CAB*a*b
    C0  = p8+..+p15
    CA  = p2+p3+p6+p7-p8-p9-p12-p13
    CB  = p4+p5+p6+p7-p8-p9-p10-p11
    CAB = p1-p2-p4-2*p6-p7+p8+2*p9+p11+p13-p14
Factored: out = ((CAB*a + CB)*b) + (CA*a + C0)  -> 6 elementwise passes.

Sharding: gates across the 8 cores (1024 each; gates are independent, each
needs x columns [g, g+1]).  Per-core inputs:
    xs [2048, 1025] = x cols [1024c .. 1024c+1024] (halo col, wraparound)
    gl [1024, 16]   = gate_logits rows for this core's gates

Coefficient prep runs in a [128 partitions, 8 gates x 16 ops] layout (exp on
ScalarE, subset reductions + combines on VectorE, all on 8-element frees so
they cost ~0.1us each), then each [128, 8] coefficient is reshaped to a
[1, 1024] row by a small SBUF->SBUF DMA and broadcast to a [128, G] PSUM tile
with K=1 matmuls (ones x row).  CAB/CB are finalized first so the main loop
starts as early as possible.

Engine assignment (measured port-sharing rule: GPSIMD's SBUF port is
VectorE's rd1, so GP only contends with DVE instructions whose BOTH tensor
operands live in SBUF — and DVE/GP running 2-port-DVE + GP concurrently is
net-negative):
    VectorE: u = a*R_cab, u += R_cb, v = a*R_ca, v += R_c0   (rd0 + PSUM)
    GPSIMD:  w = u*b, o = w + v                              (pure SBUF)
VectorE runs MEGA=2 batch tiles per instruction (3D APs + step-0 broadcast on
the coefficient operand) to amortize fixed costs; GPSIMD keeps flat 2D
per-subtile APs (3D APs are ~20% slower on the Q7s).
"""

import numpy as np

NUM_GATES = 8192
INPUT_SIZE = 8192
BATCH = 2048
N_CORES = 8
G = NUM_GATES // N_CORES  # 1024 local gates
P = 128
MEGA = 2

_CACHE = {}


def _build_nc(reps=1, mega=MEGA, warm=False, rows_on_act=False, substore=False, bulk_on_act=False, inplace_o=False, first1=True, xb=4, uvb=4, wob=3, chunk0=False, swap_add=True, swap_mul=False):
    from contextlib import ExitStack

    import concourse.bacc as bacc
    import concourse.mybir as mybir
    from concourse.mybir import AluOpType as Op
    from concourse.tile import TileContext

    f32 = mybir.dt.float32
    Ax = mybir.AxisListType
    Act = mybir.ActivationFunctionType

    nc = bacc.Bacc("TRN2", target_bir_lowering=False, debug=False,
                   num_devices=N_CORES)
    xs = nc.dram_tensor("xs", [BATCH, G + 1], f32, kind="ExternalInput").ap()
    gl = nc.dram_tensor("gl", [G, 16], f32, kind="ExternalInput").ap()
    out = nc.dram_tensor("out", [BATCH, G], f32, kind="ExternalOutput").ap()

    with TileContext(nc) as tc, ExitStack() as ctx:
        cpool = ctx.enter_context(tc.tile_pool(name="coef", bufs=1))
        rpool = ctx.enter_context(tc.tile_pool(name="rows", bufs=1))
        ppool = ctx.enter_context(tc.tile_pool(name="psum", bufs=1, space="PSUM"))
        xpool = ctx.enter_context(tc.tile_pool(name="x", bufs=xb))
        upool = ctx.enter_context(tc.tile_pool(name="tu", bufs=uvb))
        vpool = ctx.enter_context(tc.tile_pool(name="tv", bufs=uvb))
        wpool = ctx.enter_context(tc.tile_pool(name="tw", bufs=wob))
        opool = ctx.enter_context(tc.tile_pool(name="o", bufs=wob))

        row_dma = nc.scalar.dma_start if rows_on_act else nc.sync.dma_start
        bulk_dma = nc.scalar.dma_start if bulk_on_act else nc.sync.dma_start

        for rep in range(reps):
            # ---- coefficients in [128 partitions, 8 gates x 16 ops] ----
            lg = cpool.tile([P, 8 * 16], f32, name=f"lg{rep}")
            row_dma(out=lg[:, :], in_=gl.rearrange("(p n) o -> p (n o)", p=P))
            E = cpool.tile([P, 8 * 16], f32, name=f"E{rep}")
            nc.scalar.activation(E[:, :], lg[:, :], Act.Exp)
            E3 = E[:, :].rearrange("p (n o) -> p n o", o=16)

            def red(sl, name):
                t = cpool.tile([P, 8], f32, name=name)
                nc.vector.tensor_reduce(t[:, :], sl, Ax.X, Op.add)
                return t

            def Eo(o):
                return E3[:, :, o]

            den = red(E3[:, :, 0:16], f"den{rep}")
            rden = cpool.tile([P, 8], f32, name=f"rden{rep}")
            nc.vector.reciprocal(rden[:, :], den[:, :])

            ones = rpool.tile([1, P], f32, name=f"ones{rep}")
            nc.vector.memset(ones[:, :], 1.0)

            R = {nm: ppool.tile([P, G], f32, name=f"R_{nm}{rep}")
                 for nm in ("cab", "cb", "ca", "c0")}
            if warm:
                nc.tensor.matmul(R["c0"][:, 0:P], ones[:, :], ones[:, :],
                                 start=True, stop=True)

            def finalize(nm, numer):
                c = cpool.tile([P, 8], f32, name=f"c_{nm}{rep}")
                nc.vector.tensor_tensor(c[:, :], numer[:, :], rden[:, :], Op.mult)
                row = rpool.tile([1, G], f32, name=f"row_{nm}{rep}")
                row_dma(out=row[:, :], in_=c[:, :])
                for j in range(0, G, 512):
                    nc.tensor.matmul(R[nm][:, j:j + 512], ones[:, :],
                                     row[:, j:j + 512], start=True, stop=True)

            # CAB = p1-p2-p4-2*p6-p7+p8+2*p9+p11+p13-p14  (needed first)
            nab = cpool.tile([P, 8], f32, name=f"nab{rep}")
            nc.vector.scalar_tensor_tensor(nab[:, :], Eo(6), -2.0, Eo(1), Op.mult, Op.add)
            t2 = cpool.tile([P, 8], f32, name=f"t2{rep}")
            nc.vector.scalar_tensor_tensor(t2[:, :], Eo(9), 2.0, Eo(8), Op.mult, Op.add)
            nc.vector.tensor_tensor(nab[:, :], nab[:, :], t2[:, :], Op.add)
            nc.vector.tensor_tensor(t2[:, :], Eo(11), Eo(13), Op.add)
            nc.vector.tensor_tensor(nab[:, :], nab[:, :], t2[:, :], Op.add)
            nc.vector.tensor_tensor(t2[:, :], Eo(2), Eo(4), Op.add)
            nc.vector.tensor_tensor(t2[:, :], t2[:, :], Eo(7), Op.add)
            nc.vector.tensor_tensor(t2[:, :], t2[:, :], Eo(14), Op.add)
            nc.vector.tensor_tensor(nab[:, :], nab[:, :], t2[:, :], Op.subtract)
            finalize("cab", nab)

            # CB = p4+p5+p6+p7-p8-p9-p10-p11 (second: completes u-chain inputs)
            pb1 = red(E3[:, :, 4:8], f"pb1{rep}")
            pb2 = red(E3[:, :, 8:12], f"pb2{rep}")
            nb = cpool.tile([P, 8], f32, name=f"nb{rep}")
            nc.vector.tensor_tensor(nb[:, :], pb1[:, :], pb2[:, :], Op.subtract)
            finalize("cb", nb)

            # CA = p2+p3+p6+p7-p8-p9-p12-p13
            pa1 = red(E3[:, :, 2:4], f"pa1{rep}")
            pa2 = red(E3[:, :, 6:8], f"pa2{rep}")
            pa3 = red(E3[:, :, 8:10], f"pa3{rep}")
            pa4 = red(E3[:, :, 12:14], f"pa4{rep}")
            na = cpool.tile([P, 8], f32, name=f"na{rep}")
            nc.vector.tensor_tensor(na[:, :], pa1[:, :], pa2[:, :], Op.add)
            nc.vector.tensor_tensor(na[:, :], na[:, :], pa3[:, :], Op.subtract)
            nc.vector.tensor_tensor(na[:, :], na[:, :], pa4[:, :], Op.subtract)
            finalize("ca", na)

            # C0 = p8+..+p15
            n0 = red(E3[:, :, 8:16], f"n0{rep}")
            finalize("c0", n0)

            def bc(r, m):
                return r[:, :].unsqueeze(1).broadcast_to([P, m, G])

            # ---- main loop ----
            if chunk0:
                sizes = [1, 1] + [mega] * ((BATCH // P - 4) // mega) + [1, 1]
            elif first1:
                sizes = [1] + [mega] * ((BATCH // P - 2) // mega) + [1]
            else:
                sizes = [mega] * (BATCH // (P * mega))
            assert sum(sizes) == BATCH // P
            rows_lo = 0
            for gi, m in enumerate(sizes):
                xin = xs[rows_lo:rows_lo + P * m, :].rearrange(
                    "(m p) c -> p m c", m=m)
                rows_next = rows_lo + P * m
                xt = xpool.tile([P, m, G + 1], f32, name=f"xt{rep}_{gi}", tag="xt")
                bulk_dma(out=xt[:, :, :], in_=xin)
                a = xt[:, :, 0:G]

                u = upool.tile([P, m, G], f32, name=f"u{rep}_{gi}", tag="u")
                v = vpool.tile([P, m, G], f32, name=f"v{rep}_{gi}", tag="v")
                w = wpool.tile([P, m, G], f32, name=f"w{rep}_{gi}", tag="w")
                o = w if inplace_o else opool.tile([P, m, G], f32,
                                                   name=f"o{rep}_{gi}", tag="o")
                if chunk0 and gi < 2:
                    # group 0 in 512-col halves: each half depends only on the
                    # matching 512-col broadcast chunks, so the GPSIMD stream
                    # starts ~4us earlier
                    x2, u2, v2 = xt[:, 0, :], u[:, 0, :], v[:, 0, :]
                    w2, o2 = w[:, 0, :], o[:, 0, :]
                    for h in (0, 512):
                        hs = slice(h, h + 512)
                        nc.vector.tensor_tensor(u2[:, hs], x2[:, hs],
                                                R["cab"][:, hs], Op.mult)
                        nc.vector.tensor_tensor(u2[:, hs], u2[:, hs],
                                                R["cb"][:, hs], Op.add)
                        nc.vector.tensor_tensor(v2[:, hs], x2[:, hs],
                                                R["ca"][:, hs], Op.mult)
                        nc.vector.tensor_tensor(v2[:, hs], v2[:, hs],
                                                R["c0"][:, hs], Op.add)
                        nc.gpsimd.tensor_tensor(w2[:, hs], u2[:, hs],
                                                x2[:, h + 1:h + 513], Op.mult)
                        nc.gpsimd.tensor_tensor(o2[:, hs], w2[:, hs],
                                                v2[:, hs], Op.add)
                else:
                    nc.vector.tensor_tensor(u[:, :, :], a, bc(R["cab"], m), Op.mult)
                    nc.vector.tensor_tensor(u[:, :, :], u[:, :, :], bc(R["cb"], m), Op.add)
                    nc.vector.tensor_tensor(v[:, :, :], a, bc(R["ca"], m), Op.mult)
                    nc.vector.tensor_tensor(v[:, :, :], v[:, :, :], bc(R["c0"], m), Op.add)
                    for sm in range(m):
                        if swap_mul:
                            nc.gpsimd.tensor_tensor(w[:, sm, :],
                                                    xt[:, sm, 1:G + 1],
                                                    u[:, sm, :], Op.mult)
                        else:
                            nc.gpsimd.tensor_tensor(w[:, sm, :], u[:, sm, :],
                                                    xt[:, sm, 1:G + 1], Op.mult)
                        if swap_add:
                            nc.gpsimd.tensor_tensor(o[:, sm, :], v[:, sm, :],
                                                    w[:, sm, :], Op.add)
                        else:
                            nc.gpsimd.tensor_tensor(o[:, sm, :], w[:, sm, :],
                                                    v[:, sm, :], Op.add)
                if substore:
                    for sm in range(m):
                        nc.sync.dma_start(
                            out=out[rows_lo + sm * P:rows_lo + (sm + 1) * P, :],
                            in_=o[:, sm, :])
                if not substore:
                    oout = out[rows_lo:rows_lo + P * m, :].rearrange(
                        "(m p) c -> p m c", m=m)
                    nc.sync.dma_start(out=oout, in_=o[:, :, :])
                rows_lo = rows_next

    nc.compile()
    return nc


def _get_nc(reps=1, **kw):
    key = (reps, tuple(sorted(kw.items())))
    if key not in _CACHE:
        _CACHE[key] = _build_nc(reps, **kw)
    return _CACHE[key]


def _shard_inputs(x, gate_logits):
    x = np.ascontiguousarray(x, dtype=np.float32)
    gate_logits = np.ascontiguousarray(gate_logits, dtype=np.float32)
    xs_full = np.concatenate([x, x[:, :1]], axis=1)  # wraparound halo
    in_maps = []
    for c in range(N_CORES):
        in_maps.append({
            "xs": np.ascontiguousarray(xs_full[:, c * G:c * G + G + 1]),
            "gl": np.ascontiguousarray(gate_logits[c * G:(c + 1) * G]),
        })
    return in_maps


def kernel(x, gate_logits):
    from concourse.bass_utils import run_bass_kernel_spmd

    nc = _get_nc()
    in_maps = _shard_inputs(x, gate_logits)
    res = run_bass_kernel_spmd(nc, in_maps, core_ids=list(range(N_CORES)))
    return np.concatenate([res.results[c]["out"] for c in range(N_CORES)], axis=1)
